# revision 7
# baseline (speedup 1.0000x reference)
"""ContextNorm (row-wise layernorm w/ ddof=1 + diag scale + bias) on 8 TRN2 cores.

out = (X - mean(X, axis=1)) / std(X, axis=1, ddof=1) * weights + bias

Sharding: data-parallel over the batch axis (65536 rows -> 8192 rows/core);
weights/bias replicated. Per core: 64 tiles of [128 partitions x 1024 free],
bn_stats/bn_aggr for mean+var, two fused scalar_tensor_tensor passes to apply
(x - m) * w then (* rstd) + b.
"""

import sys

sys.path.insert(0, "/opt/trn_rl_repo")

import numpy as np

import concourse.bass as bass
import concourse.bacc as bacc
import concourse.tile as tile
from concourse import mybir
from concourse.bass_utils import run_bass_kernel_spmd

N_CORES = 8
BATCH = 65536
D = 1024
ROWS = BATCH // N_CORES  # 8192 rows per core
P = 128
N_TILES = ROWS // P  # 64
F32 = mybir.dt.float32
DDOF_SCALE = float(D) / float(D - 1)  # biased var -> unbiased (ddof=1)

_nc_cache = None


def _broadcast_ap(ap: bass.AP, p: int) -> bass.AP:
    # Replicate a 1-D [D] DRAM tensor across p partitions (step-0 leading dim).
    return bass.AP(tensor=ap.tensor, offset=ap.offset, ap=[[0, p], *ap.ap])


def _build_nc() -> bass.Bass:
    nc = bacc.Bacc("TRN2", target_bir_lowering=False)
    X = nc.dram_tensor("X", [ROWS, D], F32, kind="ExternalInput")
    W = nc.dram_tensor("W", [D], F32, kind="ExternalInput")
    B = nc.dram_tensor("B", [D], F32, kind="ExternalInput")
    O = nc.dram_tensor("out", [ROWS, D], F32, kind="ExternalOutput")

    x_t = X[:, :].rearrange("(n p) d -> n p d", p=P)
    o_t = O[:, :].rearrange("(n p) d -> n p d", p=P)

    with tile.TileContext(nc) as tc:
        with (
            tc.tile_pool(name="consts", bufs=1) as consts,
            tc.tile_pool(name="xio", bufs=4) as xio,
            tc.tile_pool(name="stats", bufs=8) as stats,
        ):
            Wb = consts.tile([P, D], F32)
            Bb = consts.tile([P, D], F32)
            nc.gpsimd.dma_start(out=Wb, in_=_broadcast_ap(W[:], P))
            nc.gpsimd.dma_start(out=Bb, in_=_broadcast_ap(B[:], P))
            # Absorb the broadcast-DMA completion waits on DVE once, so the
            # per-tile ops below never carry a DMA wait themselves (compute
            # instruction encodings only fit a single sync wait).
            wscr = consts.tile([P, 1], F32)
            nc.vector.tensor_copy(out=wscr, in_=Wb[:, 0:1])
            nc.vector.tensor_copy(out=wscr, in_=Bb[:, 0:1])

            for i in range(N_TILES):
                xt = xio.tile([P, D], F32, tag="x")
                nc.sync.dma_start(out=xt, in_=x_t[i])

                st = stats.tile([P, 2, 6], F32, tag="bnst")
                nc.vector.bn_stats(out=st[:, 0, :], in_=xt[:, 0:512])
                nc.vector.bn_stats(out=st[:, 1, :], in_=xt[:, 512:1024])
                mv = stats.tile([P, 2], F32, tag="mv")
                nc.vector.bn_aggr(out=mv, in_=st)

                std = stats.tile([P, 1], F32, tag="std")
                nc.scalar.activation(
                    out=std,
                    in_=mv[:, 1:2],
                    func=mybir.ActivationFunctionType.Sqrt,
                    scale=DDOF_SCALE,
                )
                rstd = stats.tile([P, 1], F32, tag="rstd")
                nc.vector.reciprocal(out=rstd, in_=std)

                # x = (x - m) * rstd  (normalize in place)
                nc.vector.tensor_scalar(
                    out=xt,
                    in0=xt,
                    scalar1=mv[:, 0:1],
                    scalar2=rstd,
                    op0=mybir.AluOpType.subtract,
                    op1=mybir.AluOpType.mult,
                )
                # x *= weights; x += bias (broadcast along partitions, in place)
                nc.vector.tensor_mul(out=xt, in0=xt, in1=Wb)
                nc.vector.tensor_add(out=xt, in0=xt, in1=Bb)
                nc.sync.dma_start(out=o_t[i], in_=xt)

    nc.compile()
    return nc


def _get_nc() -> bass.Bass:
    global _nc_cache
    if _nc_cache is None:
        _nc_cache = _build_nc()
    return _nc_cache


def run(X, weights, bias, **spmd_kwargs):
    """Run on 8 cores; returns (full_output, BassKernelResults)."""
    X = np.ascontiguousarray(X, dtype=np.float32)
    w = np.ascontiguousarray(weights, dtype=np.float32)
    b = np.ascontiguousarray(bias, dtype=np.float32)
    assert X.shape == (BATCH, D) and w.shape == (D,) and b.shape == (D,)

    nc = _get_nc()
    shards = np.split(X, N_CORES, axis=0)
    in_maps = [{"X": shards[c], "W": w, "B": b} for c in range(N_CORES)]
    res = run_bass_kernel_spmd(nc, in_maps, core_ids=list(range(N_CORES)), **spmd_kwargs)
    out = np.concatenate([res.results[c]["out"] for c in range(N_CORES)], axis=0)
    return out, res


def kernel(X, weights, bias):
    out, _ = run(X, weights, bias)
    return out


# revision 9
# speedup vs baseline: 1.1827x; 1.1827x over previous
"""ContextNorm (row-wise layernorm w/ ddof=1 + diag scale + bias) on 8 TRN2 cores.

out = (X - mean(X, axis=1)) / std(X, axis=1, ddof=1) * weights + bias

Sharding: data-parallel over the batch axis (65536 rows -> 8192 rows/core);
weights/bias replicated. Per core: 64 tiles of [128 partitions x 1024 free],
bn_stats/bn_aggr for mean+var, two fused scalar_tensor_tensor passes to apply
(x - m) * w then (* rstd) + b.
"""

import sys

sys.path.insert(0, "/opt/trn_rl_repo")

import numpy as np

import concourse.bass as bass
import concourse.bacc as bacc
import concourse.tile as tile
from concourse import mybir
from concourse.bass_utils import run_bass_kernel_spmd

N_CORES = 8
BATCH = 65536
D = 1024
ROWS = BATCH // N_CORES  # 8192 rows per core
P = 128
N_TILES = ROWS // P  # 64
F32 = mybir.dt.float32
DDOF_SCALE = float(D) / float(D - 1)  # biased var -> unbiased (ddof=1)

_nc_cache = None


def _broadcast_ap(ap: bass.AP, p: int) -> bass.AP:
    # Replicate a 1-D [D] DRAM tensor across p partitions (step-0 leading dim).
    return bass.AP(tensor=ap.tensor, offset=ap.offset, ap=[[0, p], *ap.ap])


def _build_nc() -> bass.Bass:
    nc = bacc.Bacc("TRN2", target_bir_lowering=False)
    X = nc.dram_tensor("X", [ROWS, D], F32, kind="ExternalInput")
    W = nc.dram_tensor("W", [D], F32, kind="ExternalInput")
    B = nc.dram_tensor("B", [D], F32, kind="ExternalInput")
    O = nc.dram_tensor("out", [ROWS, D], F32, kind="ExternalOutput")

    x_t = X[:, :].rearrange("(n p) d -> n p d", p=P)
    o_t = O[:, :].rearrange("(n p) d -> n p d", p=P)

    with tile.TileContext(nc) as tc:
        with (
            tc.tile_pool(name="consts", bufs=1) as consts,
            tc.tile_pool(name="xio", bufs=4) as xio,
            tc.tile_pool(name="tmp", bufs=4) as tmp,
            tc.tile_pool(name="oio", bufs=4) as oio,
            tc.tile_pool(name="stats", bufs=8) as stats,
        ):
            Wb = consts.tile([P, D], F32)
            Bb = consts.tile([P, D], F32)
            nc.gpsimd.dma_start(out=Wb, in_=_broadcast_ap(W[:], P))
            nc.gpsimd.dma_start(out=Bb, in_=_broadcast_ap(B[:], P))

            for i in range(N_TILES):
                xt = xio.tile([P, D], F32, tag="x")
                nc.sync.dma_start(out=xt, in_=x_t[i])

                st = stats.tile([P, 2, 6], F32, tag="bnst")
                nc.vector.bn_stats(out=st[:, 0, :], in_=xt[:, 0:512])
                nc.vector.bn_stats(out=st[:, 1, :], in_=xt[:, 512:1024])
                mv = stats.tile([P, 2], F32, tag="mv")
                nc.vector.bn_aggr(out=mv, in_=st)

                std = stats.tile([P, 1], F32, tag="std")
                nc.scalar.activation(
                    out=std,
                    in_=mv[:, 1:2],
                    func=mybir.ActivationFunctionType.Sqrt,
                    scale=DDOF_SCALE,
                )
                rstd = stats.tile([P, 1], F32, tag="rstd")
                nc.vector.reciprocal(out=rstd, in_=std)
                # nmr = -mean * rstd
                nmr = stats.tile([P, 1], F32, tag="nmr")
                nc.vector.tensor_scalar(
                    out=nmr,
                    in0=mv[:, 0:1],
                    scalar1=rstd,
                    scalar2=-1.0,
                    op0=mybir.AluOpType.mult,
                    op1=mybir.AluOpType.mult,
                )

                # t = x * rstd + (-m * rstd)  (normalize on ACT)
                t = tmp.tile([P, D], F32, tag="t")
                nc.scalar.activation(
                    out=t,
                    in_=xt,
                    func=mybir.ActivationFunctionType.Identity,
                    bias=nmr,
                    scale=rstd,
                )
                # t *= w  (DVE)
                nc.vector.tensor_mul(out=t, in0=t, in1=Wb)
                # out = t + b  (GPSIMD; offloads DVE)
                ot = oio.tile([P, D], F32, tag="o")
                nc.gpsimd.tensor_add(out=ot, in0=t, in1=Bb)
                nc.sync.dma_start(out=o_t[i], in_=ot)

    nc.compile()
    return nc


def _get_nc() -> bass.Bass:
    global _nc_cache
    if _nc_cache is None:
        _nc_cache = _build_nc()
    return _nc_cache


def run(X, weights, bias, **spmd_kwargs):
    """Run on 8 cores; returns (full_output, BassKernelResults)."""
    X = np.ascontiguousarray(X, dtype=np.float32)
    w = np.ascontiguousarray(weights, dtype=np.float32)
    b = np.ascontiguousarray(bias, dtype=np.float32)
    assert X.shape == (BATCH, D) and w.shape == (D,) and b.shape == (D,)

    nc = _get_nc()
    shards = np.split(X, N_CORES, axis=0)
    in_maps = [{"X": shards[c], "W": w, "B": b} for c in range(N_CORES)]
    res = run_bass_kernel_spmd(nc, in_maps, core_ids=list(range(N_CORES)), **spmd_kwargs)
    out = np.concatenate([res.results[c]["out"] for c in range(N_CORES)], axis=0)
    return out, res


def kernel(X, weights, bias):
    out, _ = run(X, weights, bias)
    return out
